# revision 77
# baseline (speedup 1.0000x reference)
"""Trainium2 Bass kernel for nn_LoadPathLoss.

reference computation:
  structure = state[:, ch]                  # [B=4, D=64, H=128, W=128]
  s = structure[:, 0]
  for z in 1..63:  s = max(s, min(structure[:, z], maxpool3x3(s)))
  return relu(structure - s[:, None]).mean()

Strategy: pure data parallel over B=4 on 4 NeuronCores, one batch element per
core.  The whole scan runs in fp16 (max/min do not accumulate rounding; vs the
f32 reference the final mean agrees to ~1e-5, gate is 2e-2):
  - H-direction max3 via two PE shift-matmuls (fp16 = 1 cycle/row vs 4 for
    f32) into separate PSUM tiles, so the first combine waits only on the
    up-matmul.  The shift matrices duplicate the boundary row (synthesized
    on-chip from iotas as k == clamp(m±1)), a no-op under max, so no -inf
    offset handling is needed at the H borders.
  - combine + W-direction max3 + step min as tensor_tensor ops on DVE (fp16
    packed operands hit the 2x DVE rate; scalar_tensor_tensor would fall to
    1x since the stt variant has no DVE perf modes).
  - m = max(S, c_z) issues on DVE during the PE matmul window
    (uses max(S, min(c, below)) == min(below, max(S, c)), valid since
    below = maxpool3x3(S) >= S).
  - after nc.compile(), _strip_self_waits removes Tile's same-engine
    self-semaphore waits (engine program order already serializes them),
    which removes the ~95ns ack+sem-prop cost from every DVE chain hop.
The final mean uses relu(c - s) = max(c, s) - s over one contiguous [H, D, W]
c tile: DVE computes the maxes in place at the 2x fp16 rate while the
otherwise-idle PE reduces them with ones-vector matmuls (contraction over
partitions, fp32-accumulated into one [1, 512] PSUM bucket row; the host only
needs totals, so bucketing by column is fine); a DVE copy stages the buckets
plus the matmul-reduced s-sums through SBUF and the host combines in f64.
"""

import numpy as np

B, C, D, H, W = 4, 8, 64, 128, 128
NCORES = 4
ZCHUNK = 8          # z-slices per DMA chunk / phase-2 op
NCHUNK = D // ZCHUNK
NEG = -60000.0      # -inf stand-in well inside fp16 range

_cached = {}


def _build_nc(d_steps=D, phase2=True, psum_fp16=False, act_chunks=5):
    import concourse.bacc as bacc
    import concourse.mybir as mybir
    from concourse.tile import TileContext

    fp16 = mybir.dt.float16
    fp32 = mybir.dt.float32
    PSUM_DT = fp16 if psum_fp16 else fp32
    mx = mybir.AluOpType.max
    mn = mybir.AluOpType.min
    byp = mybir.AluOpType.bypass

    nc = bacc.Bacc("TRN2", target_bir_lowering=False, debug=False)
    cb = nc.dram_tensor("cb", [D, H, W], fp16, kind="ExternalInput")
    out = nc.dram_tensor("out", [1, 512 + W], fp32, kind="ExternalOutput")

    with TileContext(nc) as tc:
        with (
            tc.tile_pool(name="sbuf", bufs=1) as pool,
            tc.tile_pool(name="psum", bufs=2, space="PSUM") as psum,
            tc.tile_pool(name="psacc", bufs=1, space="PSUM") as psacc,
        ):
            sh = pool.tile([H, 2 * H], fp16, tag="sh")
            iot = pool.tile([H, H], mybir.dt.int32, tag="iot")
            iotc = pool.tile([H, H], mybir.dt.int32, tag="iotc")
            iotp = pool.tile([H, 1], fp32, tag="iotp")
            call = pool.tile([H, D, W], fp16, tag="call")
            S = pool.tile([H, W], fp16, tag="S")
            m = pool.tile([H, W], fp16, tag="m")
            hp = pool.tile([H, W + 2], fp16, tag="hp")
            u = pool.tile([H, W + 1], fp16, tag="u")
            below = pool.tile([H, W], fp16, tag="below")
            ones = pool.tile([H, 1], fp16, tag="ones")
            res = pool.tile([1, 512 + W], fp32, tag="res")

            # first two z-slices land first so the scan can start early
            nc.sync.dma_start(
                out=call[:, 0:2, :],
                in_=cb[0:2].rearrange("z h w -> h z w"),
            )

            # synthesize the shift matrices on-chip.  up-shift lhsT has
            # U[k, m] = 1 iff k == min(m+1, 127); dn-shift lhsT has
            # D[k, m] = 1 iff k == max(m-1, 0) — the clamp duplicates the
            # boundary row, which is a no-op under max.
            nc.gpsimd.iota(iot[:], pattern=[[1, H]], base=0, channel_multiplier=0)
            nc.gpsimd.iota(
                iotp[:], pattern=[[1, 1]], base=0, channel_multiplier=1,
                allow_small_or_imprecise_dtypes=True,
            )
            nc.vector.tensor_scalar(
                out=iotc[:], in0=iot[:], scalar1=1, scalar2=H - 1,
                op0=mybir.AluOpType.add, op1=mybir.AluOpType.min,
            )
            nc.vector.tensor_scalar(
                out=sh[:, 0:H], in0=iotc[:], scalar1=iotp[:], scalar2=0,
                op0=mybir.AluOpType.subtract, op1=mybir.AluOpType.is_equal,
            )
            nc.vector.tensor_scalar(
                out=iotc[:], in0=iot[:], scalar1=1, scalar2=0,
                op0=mybir.AluOpType.subtract, op1=mybir.AluOpType.max,
            )
            nc.vector.tensor_scalar(
                out=sh[:, H : 2 * H], in0=iotc[:], scalar1=iotp[:], scalar2=0,
                op0=mybir.AluOpType.subtract, op1=mybir.AluOpType.is_equal,
            )

            # input chunks: cb[z,h,w] -> sbuf [h, z, w]
            nc.sync.dma_start(
                out=call[:, 2:ZCHUNK, :],
                in_=cb[2:ZCHUNK].rearrange("z h w -> h z w"),
            )
            for k in range(1, NCHUNK):
                zsrc = cb[k * ZCHUNK : (k + 1) * ZCHUNK].rearrange("z h w -> h z w")
                nc.sync.dma_start(out=call[:, k * ZCHUNK : (k + 1) * ZCHUNK, :], in_=zsrc)

            # -inf-pad border columns of hp once; center overwritten each step
            nc.vector.memset(hp[:], NEG)
            nc.vector.memset(ones[:], 1.0)

            for z in range(1, d_steps):
                k, j = z // ZCHUNK, z % ZCHUNK
                c_z = call[:, z, :]
                # step 1 reads c_0 directly instead of a copied-in S
                Sr = call[:, 0, :] if z == 1 else S[:]
                # separate PSUM tiles per shift so the first combine op waits
                # only on the up-matmul, hiding the dn-matmul behind it
                ps_u = psum.tile([H, W], PSUM_DT, tag="psu", name=f"psu{z}")
                ps_d = psum.tile([H, W], PSUM_DT, tag="psd", name=f"psd{z}")
                nc.tensor.matmul(
                    out=ps_u[:], lhsT=sh[:, 0:H], rhs=Sr,
                    start=True, stop=True,
                )
                nc.tensor.matmul(
                    out=ps_d[:], lhsT=sh[:, H : 2 * H], rhs=Sr,
                    start=True, stop=True,
                )
                # m = max(S, c_z) on DVE, hidden under the PE shift window
                nc.vector.tensor_tensor(out=m[:], in0=c_z, in1=Sr, op=mx)
                # hp center = max(S, up, dn); boundary rows are duplicated by
                # the shift matrices so no offset trick is needed.  Two ops,
                # each reading one PSUM input (ISA limit).
                nc.vector.tensor_tensor(
                    out=hp[:, 1 : W + 1], in0=Sr, in1=ps_u[:], op=mx,
                )
                nc.vector.tensor_tensor(
                    out=hp[:, 1 : W + 1], in0=hp[:, 1 : W + 1],
                    in1=ps_d[:], op=mx,
                )
                # W-direction max3 on the -inf-padded hp
                nc.vector.tensor_tensor(
                    out=u[:], in0=hp[:, 0 : W + 1], in1=hp[:, 1 : W + 2], op=mx,
                )
                nc.vector.tensor_tensor(
                    out=below[:], in0=u[:, 0:W], in1=hp[:, 2 : W + 2], op=mx,
                )
                nc.vector.tensor_tensor(
                    out=S[:], in0=below[:], in1=m[:], op=mn,
                )

            # phase 2: the host only needs the TOTAL of max(c, s) and of s, so
            # the reductions run as ones-vector matmuls on the otherwise-idle
            # PE (contraction over partitions, accumulated into one PSUM
            # bucket row); DVE only computes the maxes at the 2x fp16 rate.
            ps_s = psacc.tile([1, W], fp32, tag="pss")
            nc.tensor.matmul(
                out=ps_s[:], lhsT=ones[:], rhs=S[:], start=True, stop=True,
            )
            if phase2:
                ps_a = psacc.tile([1, 512], fp32, tag="psa")
                # shrink the last max blocks so the final accumulate matmul
                # (and the PSUM copy-out behind it) starts as early as possible
                blocks = [(8 * b, 8 * b + 8) for b in range(7)]
                blocks += [(56, 60), (60, 63), (63, 64)]
                mms = []
                for (a, b) in blocks:
                    for h in range(a, b, 4):
                        mms.append((h, min(h + 4, b)))
                mi, nmm = 0, len(mms)
                for (a, b) in blocks:
                    sbc = S[:].unsqueeze(1).broadcast_to((H, b - a, W))
                    nc.vector.tensor_tensor(
                        out=call[:, a:b, :], in0=call[:, a:b, :],
                        in1=sbc, op=mx,
                    )
                    while mi < nmm and mms[mi][1] <= b:
                        h0, h1 = mms[mi]
                        nc.tensor.matmul(
                            out=ps_a[:, 0 : (h1 - h0) * W], lhsT=ones[:],
                            rhs=call[:, h0:h1, :],
                            start=(mi == 0), stop=(mi == nmm - 1),
                        )
                        mi += 1

                nc.vector.tensor_copy(res[0:1, 0:512], ps_a[:])
            nc.vector.tensor_copy(res[0:1, 512 : 512 + W], ps_s[:])
            nc.sync.dma_start(out=out[:, :], in_=res[:])

    nc.compile()
    _strip_self_waits(nc)
    return nc


def _strip_self_waits(nc):
    """Drop semaphore waits where an instruction waits on a sem that is only
    ever incremented by earlier non-DMA instructions of its own engine —
    engine program order already guarantees those.  Cross-engine and
    DMA-completion waits are kept."""
    import concourse.mybir as mybir

    fn = nc.m.functions[0]
    insts = [i for bb in fn.blocks for i in bb.instructions]

    dma_ops = ("DMACopy", "TensorLoad", "TensorSave", "TriggeredCopy")
    updaters = {}   # sem name -> set of engines with non-DMA updates
    dma_sems = set()
    for inst in insts:
        si = inst.sync_info
        if not si:
            continue
        for u in (si.on_update or []):
            name = u.ant_name
            is_dma = inst.opcode in dma_ops
            if is_dma:
                dma_sems.add(name)
            else:
                updaters.setdefault(name, set()).add(inst.engine)

    n_stripped = 0
    for inst in insts:
        si = inst.sync_info
        if not si or not si.on_wait:
            continue
        if inst.opcode in dma_ops:
            # a DMA trigger fires at SEQ time; engine program order does NOT
            # order it after preceding compute — keep all its waits
            continue
        keep = []
        for w in si.on_wait:
            name = w.ant_name
            if (
                name not in dma_sems
                and updaters.get(name) == {inst.engine}
            ):
                n_stripped += 1
                continue
            keep.append(w)
        if len(keep) != len(si.on_wait):
            si.on_wait = keep


def _make_runner(nc):
    """Cached multi-core PJRT runner (mirrors bass2jax.run_bass_via_pjrt but
    keeps the jitted shard_map so repeat calls skip retrace/recompile)."""
    import jax
    from jax.sharding import Mesh, PartitionSpec
    from jax.experimental.shard_map import shard_map
    import concourse.mybir as mybir
    from concourse import bass2jax

    bass2jax.install_neuronx_cc_hook()

    partition_name = nc.partition_id_tensor.name if nc.partition_id_tensor else None
    in_names, out_names, out_avals, zero_outs = [], [], [], []
    for alloc in nc.m.functions[0].allocations:
        if not isinstance(alloc, mybir.MemoryLocationSet):
            continue
        name = alloc.memorylocations[0].name
        if alloc.kind == "ExternalInput":
            if name != partition_name:
                in_names.append(name)
        elif alloc.kind == "ExternalOutput":
            shape = tuple(alloc.tensor_shape)
            dtype = mybir.dt.np(alloc.dtype)
            out_names.append(name)
            out_avals.append(jax.core.ShapedArray(shape, dtype))
            zero_outs.append(np.zeros(shape, dtype))
    n_params = len(in_names)
    n_outs = len(out_avals)
    all_names = in_names + out_names
    donate = tuple(range(n_params, n_params + n_outs))

    def _body(*args):
        operands = list(args)
        if partition_name is not None:
            operands.append(bass2jax.partition_id_tensor())
        outs = bass2jax._bass_exec_p.bind(
            *operands,
            out_avals=tuple(out_avals),
            in_names=tuple(all_names + ([partition_name] if partition_name else [])),
            out_names=tuple(out_names),
            lowering_input_output_aliases=(),
            sim_require_finite=True,
            sim_require_nnan=True,
            nc=nc,
        )
        return tuple(outs)

    devices = jax.devices()[:NCORES]
    mesh = Mesh(np.asarray(devices), ("core",))
    in_specs = (PartitionSpec("core"),) * (n_params + n_outs)
    out_specs = (PartitionSpec("core"),) * n_outs
    sharded = jax.jit(
        shard_map(_body, mesh=mesh, in_specs=in_specs, out_specs=out_specs,
                  check_rep=False),
        donate_argnums=donate, keep_unused=True,
    )

    def run(in_maps):
        args = [
            np.concatenate([np.asarray(m[name]) for m in in_maps], axis=0)
            for name in in_names
        ]
        zouts = [np.concatenate([z] * NCORES, axis=0) for z in zero_outs]
        outs = sharded(*args, *zouts)
        res = []
        for b in range(NCORES):
            d = {}
            for i, name in enumerate(out_names):
                full = np.asarray(outs[i])
                per = full.shape[0] // NCORES
                d[name] = full[b * per : (b + 1) * per]
            res.append(d)
        return res

    return run


def kernel(state, ch_structure):
    if "nc" not in _cached:
        _cached["nc"] = _build_nc()
        _cached["run"] = _make_runner(_cached["nc"])

    structure = np.ascontiguousarray(
        state[:, int(ch_structure)].astype(np.float16)
    )
    in_maps = [{"cb": structure[b]} for b in range(NCORES)]
    results = _cached["run"](in_maps)
    _cached["last"] = results

    total = 0.0
    for b in range(NCORES):
        o = results[b]["out"].astype(np.float64)
        total += o[0, :512].sum() - float(D) * o[0, 512:].sum()
    mean = total / float(B * D * H * W)
    return np.asarray(mean, dtype=np.float32)


if __name__ == "__main__":
    rng = np.random.default_rng(0)
    st = rng.standard_normal((B, C, D, H, W)).astype(np.float32)
    print(kernel(st, 3))


# revision 80
# speedup vs baseline: 1.0011x; 1.0011x over previous
"""Trainium2 Bass kernel for nn_LoadPathLoss.

reference computation:
  structure = state[:, ch]                  # [B=4, D=64, H=128, W=128]
  s = structure[:, 0]
  for z in 1..63:  s = max(s, min(structure[:, z], maxpool3x3(s)))
  return relu(structure - s[:, None]).mean()

Strategy: pure data parallel over B=4 on 4 NeuronCores, one batch element per
core.  The whole scan runs in fp16 (max/min do not accumulate rounding; vs the
f32 reference the final mean agrees to ~1e-5, gate is 2e-2):
  - H-direction max3 via two PE shift-matmuls (fp16 = 1 cycle/row vs 4 for
    f32) into separate PSUM tiles, so the first combine waits only on the
    up-matmul.  The shift matrices duplicate the boundary row (synthesized
    on-chip from iotas as k == clamp(m±1)), a no-op under max, so no -inf
    offset handling is needed at the H borders.
  - combine + W-direction max3 + step min as tensor_tensor ops on DVE (fp16
    packed operands hit the 2x DVE rate; scalar_tensor_tensor would fall to
    1x since the stt variant has no DVE perf modes).
  - m = max(S, c_z) issues on DVE during the PE matmul window
    (uses max(S, min(c, below)) == min(below, max(S, c)), valid since
    below = maxpool3x3(S) >= S).
  - after nc.compile(), _strip_self_waits removes Tile's same-engine
    self-semaphore waits (engine program order already serializes them),
    which removes the ~95ns ack+sem-prop cost from every DVE chain hop.
The final mean uses relu(c - s) = max(c, s) - s over one contiguous [H, D, W]
c tile: DVE computes the maxes in place at the 2x fp16 rate while the
otherwise-idle PE reduces them with ones-vector matmuls (contraction over
partitions, fp32-accumulated into one [1, 512] PSUM bucket row; the host only
needs totals, so bucketing by column is fine); a DVE copy stages the buckets
plus the matmul-reduced s-sums through SBUF and the host combines in f64.
"""

import numpy as np

B, C, D, H, W = 4, 8, 64, 128, 128
NCORES = 4
ZCHUNK = 8          # z-slices per DMA chunk / phase-2 op
NCHUNK = D // ZCHUNK
NEG = -60000.0      # -inf stand-in well inside fp16 range

_cached = {}


def _build_nc(d_steps=D, phase2=True, psum_fp16=False, act_chunks=5):
    import concourse.bacc as bacc
    import concourse.mybir as mybir
    from concourse.tile import TileContext

    fp16 = mybir.dt.float16
    fp32 = mybir.dt.float32
    PSUM_DT = fp16 if psum_fp16 else fp32
    mx = mybir.AluOpType.max
    mn = mybir.AluOpType.min
    byp = mybir.AluOpType.bypass

    nc = bacc.Bacc("TRN2", target_bir_lowering=False, debug=False)
    cb = nc.dram_tensor("cb", [D, H, W], fp16, kind="ExternalInput")
    out = nc.dram_tensor("out", [1, 512 + W], fp32, kind="ExternalOutput")

    with TileContext(nc) as tc:
        with (
            tc.tile_pool(name="sbuf", bufs=1) as pool,
            tc.tile_pool(name="psum", bufs=2, space="PSUM") as psum,
            tc.tile_pool(name="psacc", bufs=1, space="PSUM") as psacc,
        ):
            sh = pool.tile([H, 2 * H], fp16, tag="sh")
            iot = pool.tile([H, H], mybir.dt.int32, tag="iot")
            iotc = pool.tile([H, H], mybir.dt.int32, tag="iotc")
            iotp = pool.tile([H, 1], fp32, tag="iotp")
            call = pool.tile([H, D, W], fp16, tag="call")
            S = pool.tile([H, W], fp16, tag="S")
            m = pool.tile([H, W], fp16, tag="m")
            hp = pool.tile([H, W + 2], fp16, tag="hp")
            u = pool.tile([H, W + 1], fp16, tag="u")
            below = pool.tile([H, W], fp16, tag="below")
            ones = pool.tile([H, 1], fp16, tag="ones")
            res = pool.tile([1, 512 + W], fp32, tag="res")

            # first two z-slices land first so the scan can start early
            nc.sync.dma_start(
                out=call[:, 0:2, :],
                in_=cb[0:2].rearrange("z h w -> h z w"),
            )

            # synthesize the shift matrices on-chip.  up-shift lhsT has
            # U[k, m] = 1 iff k == min(m+1, 127); dn-shift lhsT has
            # D[k, m] = 1 iff k == max(m-1, 0) — the clamp duplicates the
            # boundary row, which is a no-op under max.
            nc.gpsimd.iota(iot[:], pattern=[[1, H]], base=0, channel_multiplier=0)
            nc.gpsimd.iota(
                iotp[:], pattern=[[1, 1]], base=0, channel_multiplier=1,
                allow_small_or_imprecise_dtypes=True,
            )
            nc.vector.tensor_scalar(
                out=iotc[:], in0=iot[:], scalar1=1, scalar2=H - 1,
                op0=mybir.AluOpType.add, op1=mybir.AluOpType.min,
            )
            nc.vector.tensor_scalar(
                out=sh[:, 0:H], in0=iotc[:], scalar1=iotp[:], scalar2=0,
                op0=mybir.AluOpType.subtract, op1=mybir.AluOpType.is_equal,
            )
            nc.vector.tensor_scalar(
                out=iotc[:], in0=iot[:], scalar1=1, scalar2=0,
                op0=mybir.AluOpType.subtract, op1=mybir.AluOpType.max,
            )
            nc.vector.tensor_scalar(
                out=sh[:, H : 2 * H], in0=iotc[:], scalar1=iotp[:], scalar2=0,
                op0=mybir.AluOpType.subtract, op1=mybir.AluOpType.is_equal,
            )

            # input chunks: cb[z,h,w] -> sbuf [h, z, w]; z2-3 ship separately
            # so step 2 is not gated by the full z2-7 transfer
            nc.sync.dma_start(
                out=call[:, 2:4, :],
                in_=cb[2:4].rearrange("z h w -> h z w"),
            )
            nc.sync.dma_start(
                out=call[:, 4:ZCHUNK, :],
                in_=cb[4:ZCHUNK].rearrange("z h w -> h z w"),
            )
            for k in range(1, NCHUNK):
                zsrc = cb[k * ZCHUNK : (k + 1) * ZCHUNK].rearrange("z h w -> h z w")
                nc.sync.dma_start(out=call[:, k * ZCHUNK : (k + 1) * ZCHUNK, :], in_=zsrc)

            # -inf-pad border columns of hp once; center overwritten each step
            nc.vector.memset(hp[:], NEG)
            nc.vector.memset(ones[:], 1.0)

            for z in range(1, d_steps):
                k, j = z // ZCHUNK, z % ZCHUNK
                c_z = call[:, z, :]
                # step 1 reads c_0 directly instead of a copied-in S
                Sr = call[:, 0, :] if z == 1 else S[:]
                # separate PSUM tiles per shift so the first combine op waits
                # only on the up-matmul, hiding the dn-matmul behind it
                ps_u = psum.tile([H, W], PSUM_DT, tag="psu", name=f"psu{z}")
                ps_d = psum.tile([H, W], PSUM_DT, tag="psd", name=f"psd{z}")
                nc.tensor.matmul(
                    out=ps_u[:], lhsT=sh[:, 0:H], rhs=Sr,
                    start=True, stop=True,
                )
                nc.tensor.matmul(
                    out=ps_d[:], lhsT=sh[:, H : 2 * H], rhs=Sr,
                    start=True, stop=True,
                )
                # m = max(S, c_z) on DVE, hidden under the PE shift window
                nc.vector.tensor_tensor(out=m[:], in0=c_z, in1=Sr, op=mx)
                # hp center = max(S, up, dn); boundary rows are duplicated by
                # the shift matrices so no offset trick is needed.  Two ops,
                # each reading one PSUM input (ISA limit).
                nc.vector.tensor_tensor(
                    out=hp[:, 1 : W + 1], in0=Sr, in1=ps_u[:], op=mx,
                )
                nc.vector.tensor_tensor(
                    out=hp[:, 1 : W + 1], in0=hp[:, 1 : W + 1],
                    in1=ps_d[:], op=mx,
                )
                # W-direction max3 on the -inf-padded hp
                nc.vector.tensor_tensor(
                    out=u[:], in0=hp[:, 0 : W + 1], in1=hp[:, 1 : W + 2], op=mx,
                )
                nc.vector.tensor_tensor(
                    out=below[:], in0=u[:, 0:W], in1=hp[:, 2 : W + 2], op=mx,
                )
                nc.vector.tensor_tensor(
                    out=S[:], in0=below[:], in1=m[:], op=mn,
                )

            # phase 2: the host only needs the TOTAL of max(c, s) and of s, so
            # the reductions run as ones-vector matmuls on the otherwise-idle
            # PE (contraction over partitions, accumulated into one PSUM
            # bucket row); DVE only computes the maxes at the 2x fp16 rate.
            ps_s = psacc.tile([1, W], fp32, tag="pss")
            nc.tensor.matmul(
                out=ps_s[:], lhsT=ones[:], rhs=S[:], start=True, stop=True,
            )
            if phase2:
                ps_a = psacc.tile([1, 512], fp32, tag="psa")
                # shrink the last max blocks so the final accumulate matmul
                # (and the PSUM copy-out behind it) starts as early as possible
                blocks = [(8 * b, 8 * b + 8) for b in range(7)]
                blocks += [(56, 60), (60, 63), (63, 64)]
                mms = []
                for (a, b) in blocks:
                    for h in range(a, b, 4):
                        mms.append((h, min(h + 4, b)))
                mi, nmm = 0, len(mms)
                for (a, b) in blocks:
                    sbc = S[:].unsqueeze(1).broadcast_to((H, b - a, W))
                    nc.vector.tensor_tensor(
                        out=call[:, a:b, :], in0=call[:, a:b, :],
                        in1=sbc, op=mx,
                    )
                    while mi < nmm and mms[mi][1] <= b:
                        h0, h1 = mms[mi]
                        nc.tensor.matmul(
                            out=ps_a[:, 0 : (h1 - h0) * W], lhsT=ones[:],
                            rhs=call[:, h0:h1, :],
                            start=(mi == 0), stop=(mi == nmm - 1),
                        )
                        mi += 1

                nc.vector.tensor_copy(res[0:1, 0:512], ps_a[:])
            nc.vector.tensor_copy(res[0:1, 512 : 512 + W], ps_s[:])
            nc.sync.dma_start(out=out[:, :], in_=res[:])

    nc.compile()
    _strip_self_waits(nc)
    return nc


def _strip_self_waits(nc):
    """Drop semaphore waits where an instruction waits on a sem that is only
    ever incremented by earlier non-DMA instructions of its own engine —
    engine program order already guarantees those.  Cross-engine and
    DMA-completion waits are kept."""
    import concourse.mybir as mybir

    fn = nc.m.functions[0]
    insts = [i for bb in fn.blocks for i in bb.instructions]

    dma_ops = ("DMACopy", "TensorLoad", "TensorSave", "TriggeredCopy")
    updaters = {}   # sem name -> set of engines with non-DMA updates
    dma_sems = set()
    for inst in insts:
        si = inst.sync_info
        if not si:
            continue
        for u in (si.on_update or []):
            name = u.ant_name
            is_dma = inst.opcode in dma_ops
            if is_dma:
                dma_sems.add(name)
            else:
                updaters.setdefault(name, set()).add(inst.engine)

    n_stripped = 0
    for inst in insts:
        si = inst.sync_info
        if not si or not si.on_wait:
            continue
        if inst.opcode in dma_ops:
            # a DMA trigger fires at SEQ time; engine program order does NOT
            # order it after preceding compute — keep all its waits
            continue
        keep = []
        for w in si.on_wait:
            name = w.ant_name
            if (
                name not in dma_sems
                and updaters.get(name) == {inst.engine}
            ):
                n_stripped += 1
                continue
            keep.append(w)
        if len(keep) != len(si.on_wait):
            si.on_wait = keep


def _make_runner(nc):
    """Cached multi-core PJRT runner (mirrors bass2jax.run_bass_via_pjrt but
    keeps the jitted shard_map so repeat calls skip retrace/recompile)."""
    import jax
    from jax.sharding import Mesh, PartitionSpec
    from jax.experimental.shard_map import shard_map
    import concourse.mybir as mybir
    from concourse import bass2jax

    bass2jax.install_neuronx_cc_hook()

    partition_name = nc.partition_id_tensor.name if nc.partition_id_tensor else None
    in_names, out_names, out_avals, zero_outs = [], [], [], []
    for alloc in nc.m.functions[0].allocations:
        if not isinstance(alloc, mybir.MemoryLocationSet):
            continue
        name = alloc.memorylocations[0].name
        if alloc.kind == "ExternalInput":
            if name != partition_name:
                in_names.append(name)
        elif alloc.kind == "ExternalOutput":
            shape = tuple(alloc.tensor_shape)
            dtype = mybir.dt.np(alloc.dtype)
            out_names.append(name)
            out_avals.append(jax.core.ShapedArray(shape, dtype))
            zero_outs.append(np.zeros(shape, dtype))
    n_params = len(in_names)
    n_outs = len(out_avals)
    all_names = in_names + out_names
    donate = tuple(range(n_params, n_params + n_outs))

    def _body(*args):
        operands = list(args)
        if partition_name is not None:
            operands.append(bass2jax.partition_id_tensor())
        outs = bass2jax._bass_exec_p.bind(
            *operands,
            out_avals=tuple(out_avals),
            in_names=tuple(all_names + ([partition_name] if partition_name else [])),
            out_names=tuple(out_names),
            lowering_input_output_aliases=(),
            sim_require_finite=True,
            sim_require_nnan=True,
            nc=nc,
        )
        return tuple(outs)

    devices = jax.devices()[:NCORES]
    mesh = Mesh(np.asarray(devices), ("core",))
    in_specs = (PartitionSpec("core"),) * (n_params + n_outs)
    out_specs = (PartitionSpec("core"),) * n_outs
    sharded = jax.jit(
        shard_map(_body, mesh=mesh, in_specs=in_specs, out_specs=out_specs,
                  check_rep=False),
        donate_argnums=donate, keep_unused=True,
    )

    def run(in_maps):
        args = [
            np.concatenate([np.asarray(m[name]) for m in in_maps], axis=0)
            for name in in_names
        ]
        zouts = [np.concatenate([z] * NCORES, axis=0) for z in zero_outs]
        outs = sharded(*args, *zouts)
        res = []
        for b in range(NCORES):
            d = {}
            for i, name in enumerate(out_names):
                full = np.asarray(outs[i])
                per = full.shape[0] // NCORES
                d[name] = full[b * per : (b + 1) * per]
            res.append(d)
        return res

    return run


def kernel(state, ch_structure):
    if "nc" not in _cached:
        _cached["nc"] = _build_nc()
        _cached["run"] = _make_runner(_cached["nc"])

    structure = np.ascontiguousarray(
        state[:, int(ch_structure)].astype(np.float16)
    )
    in_maps = [{"cb": structure[b]} for b in range(NCORES)]
    results = _cached["run"](in_maps)
    _cached["last"] = results

    total = 0.0
    for b in range(NCORES):
        o = results[b]["out"].astype(np.float64)
        total += o[0, :512].sum() - float(D) * o[0, 512:].sum()
    mean = total / float(B * D * H * W)
    return np.asarray(mean, dtype=np.float32)


if __name__ == "__main__":
    rng = np.random.default_rng(0)
    st = rng.standard_normal((B, C, D, H, W)).astype(np.float32)
    print(kernel(st, 3))
